# revision 17
# baseline (speedup 1.0000x reference)
"""GCN block (DGL GraphConv norm='both' + ReLU) on 8 TRN2 NeuronCores.

Strategy (SPMD, one program for all cores; per-core data via inputs):
  - Nodes/edges sharded by destination: core c owns dst rows [c*6250, (c+1)*6250).
  - Host folds the source norm into the feature table: xh = x * rsqrt(deg_out)
    cast to bf16 (one 256B row per node). Each core gathers xh[src] for its
    edges straight from its HBM copy with SWDGE dma_gather — no on-device
    table build; gathers start immediately. SWDGE descriptor generation on
    the Q7 (~2.3ns/row marginal, serial) is the kernel's critical path, so
    everything else is kept off it and overlapped under it.
  - Segment-sum by dst via TensorE: edges grouped by (128-wide dst block,
    src half) with a VARIABLE number of 128-edge tiles per group (max over
    cores, so the single SPMD program fits every core's data with ~6% padding
    instead of the ~32% a global max would cost). Per tile one matmul
    (lhsT = gathered rows [128e x 128f] bf16, rhs = is_equal one-hot
    [128e x 128d] bf16) accumulates the block's aggT[f, d] in PSUM; flushes
    add into a memset aggT so empty groups need no special casing.
  - Output stage is pipelined into pass B: as soon as block b's second-half
    flush lands, aggT_b x W (f32 matmul), then ReLU with the rsqrt(deg_in)
    row scale fused on the SCALAR engine (exact: the per-dst scale commutes
    through the feature matmul), DMA out. Nonzero bias falls back to vector.
  - Pass B tapers into small tail gathers on dedicated rings so the final
    rows land right after the last descriptor batch.

dma_gather indices are int16, so the table is addressed in two halves at row
32768; pass A covers half-0 edges, pass B half-1.
"""

import sys

if "/opt/trn_rl_repo" not in sys.path:
    sys.path.insert(0, "/opt/trn_rl_repo")

import numpy as np
import ml_dtypes

import concourse.bacc as bacc
import concourse.mybir as mybir
from concourse.bass_utils import run_bass_kernel_spmd
from concourse.tile import TileContext

N = 50000          # nodes
D = 128            # feature dim
NCORES = 8
NPC = N // NCORES  # 6250 dst nodes per core
RN = 50048         # padded table rows (multiple of 128)
HALF = 32768       # int16 index limit; table addressed [0, HALF) / [HALF, RN)

DST_BLK = 128                     # dst nodes per PSUM block
NBLK = (NPC + DST_BLK - 1) // DST_BLK   # 49
OCH = NBLK                        # output chunks of 128 dst rows

GCH = 40                          # gather chunk: tiles per dma_gather call
NQ = 4                            # SWDGE queues used round-robin

F32 = mybir.dt.float32
BF16 = mybir.dt.bfloat16
FP8 = mybir.dt.float8e4
I16 = mybir.dt.int16

TRACE = False            # set by test harness for profiling
LAST_RESULTS = None      # BassKernelResults of the last run


def _gather_idx_layout(vals):
    """[S] int16 -> [128, S//16] in dma_gather layout (16-wrap, 8x replicated)."""
    base = vals.reshape(-1, 16).T          # [16, S/16]
    return np.ascontiguousarray(np.tile(base, (8, 1)))


def _prep_inputs(x, edge_index, W, b):
    src = np.asarray(edge_index[0], dtype=np.int64)
    dst = np.asarray(edge_index[1], dtype=np.int64)

    deg_out = np.bincount(src, minlength=N).astype(np.float32)
    deg_in = np.bincount(dst, minlength=N).astype(np.float32)
    nsrc = 1.0 / np.sqrt(np.maximum(deg_out, 1.0))
    ndst = 1.0 / np.sqrt(np.maximum(deg_in, 1.0))

    core = dst // NPC
    half = (src >= HALF).astype(np.int64)
    dstl = dst - core * NPC                # local dst id
    blk = dstl // DST_BLK                  # 0..NBLK-1

    # group = (half, blk); edges with the same (core, grp, src) share one
    # gathered row (multi-hot one-hot column), trimming ~2% of the rows
    grp = half * NBLK + blk                # 0..2*NBLK-1
    key = core * (2 * NBLK) + grp
    pairkey = key * N + src
    order = np.argsort(pairkey, kind="stable")
    pk_s = pairkey[order]
    first = np.empty(len(src), dtype=bool)
    first[0] = True
    first[1:] = pk_s[1:] != pk_s[:-1]
    uidx = np.cumsum(first) - 1            # edge (sorted) -> unique-slot index
    ukey = key[order][first]               # per unique slot
    usrc = src[order][first]
    uhalf = half[order][first]
    ucore = core[order][first]
    ugrp = grp[order][first]

    ucounts = np.bincount(ukey, minlength=NCORES * 2 * NBLK).reshape(
        NCORES, 2 * NBLK)
    tiles_per_grp = -(-ucounts.max(axis=0) // 128)      # [2*NBLK] int
    grp_tile_start = np.zeros(2 * NBLK + 1, dtype=np.int64)
    np.cumsum(tiles_per_grp, out=grp_tile_start[1:])
    TTOT = int(grp_tile_start[-1])
    TA = int(tiles_per_grp[:NBLK].sum())   # tiles in pass A (half 0)

    # slot of each unique (core, grp, src): per (core, grp) running rank
    gstart = np.zeros(NCORES * 2 * NBLK + 1, dtype=np.int64)
    np.cumsum(ucounts.reshape(-1), out=gstart[1:])
    urank = np.arange(len(usrc), dtype=np.int64) - gstart[ukey]
    uslot = ucore * (TTOT * 128) + grp_tile_start[ugrp] * 128 + urank

    idx_all = np.zeros(NCORES * TTOT * 128, dtype=np.int16)   # pad: row 0
    idx_all[uslot] = (usrc - uhalf * HALF).astype(np.int16)

    # host multi-hot (small integer counts, exact in fp8):
    # oh[slot, dloc] += 1 per edge
    oh_f32 = np.zeros((NCORES * TTOT * 128, DST_BLK), dtype=np.float32)
    np.add.at(oh_f32, (uslot[uidx], (dstl - blk * DST_BLK)[order]), 1.0)
    oh_all = oh_f32.astype(ml_dtypes.float8_e4m3)
    del oh_f32

    # replicated tensors
    xh = np.zeros((RN, D), dtype=ml_dtypes.bfloat16)
    xh[:N] = (np.asarray(x, dtype=np.float32) * nsrc[:, None]).astype(
        ml_dtypes.bfloat16)

    W_dev = np.ascontiguousarray(np.asarray(W, dtype=np.float32))
    use_bias = bool(np.any(np.asarray(b, dtype=np.float32) != 0.0))
    brep = np.ascontiguousarray(
        np.tile(np.asarray(b, dtype=np.float32)[None, :], (128, 1)))

    in_maps = []
    idx3 = idx_all.reshape(NCORES, TTOT * 128)
    oh4 = oh_all.reshape(NCORES, TTOT, 128, DST_BLK)
    for c in range(NCORES):
        nd = np.ones(OCH * 128, dtype=np.float32)
        nd[:NPC] = ndst[c * NPC:(c + 1) * NPC]
        ndst_dev = np.ascontiguousarray(nd.reshape(OCH, 128).T)
        in_maps.append({
            "xh": xh,
            "w": W_dev,
            "brep": brep,
            "ndst": ndst_dev,
            "idx": _gather_idx_layout(idx3[c]),
            # oh tile t lane e col d -> oh_dev[e, t, d]
            "oh": np.ascontiguousarray(oh4[c].transpose(1, 0, 2)),
        })
    return in_maps, [int(t) for t in tiles_per_grp], TTOT, TA, use_bias


def _build_program(tiles_per_grp, TTOT, TA, use_bias):
    nc = bacc.Bacc("TRN2", target_bir_lowering=False, debug=False,
                   num_devices=NCORES, num_swdge_queues=NQ)

    xh_d = nc.dram_tensor("xh", [RN, D], BF16, kind="ExternalInput")
    w_d = nc.dram_tensor("w", [D, D], F32, kind="ExternalInput")
    brep_d = nc.dram_tensor("brep", [128, D], F32, kind="ExternalInput")
    oh_d = nc.dram_tensor("oh", [128, TTOT, DST_BLK], FP8,
                          kind="ExternalInput")
    ndst_d = nc.dram_tensor("ndst", [128, OCH], F32, kind="ExternalInput")
    idx_d = nc.dram_tensor("idx", [128, TTOT * 8], I16, kind="ExternalInput")
    y_d = nc.dram_tensor("y", [128, OCH, D], F32, kind="ExternalOutput")

    # per-tile metadata: (k within group, group size, blk, half)
    tmeta = []
    for g in range(2 * NBLK):
        T = tiles_per_grp[g]
        for k in range(T):
            tmeta.append((k, T, g % NBLK, g // NBLK))
    assert len(tmeta) == TTOT

    P0 = min(8, TA)    # first idx piece/call: small, so gathers start fast

    with TileContext(nc) as tc:
        with (
            tc.tile_pool(name="const", bufs=1) as cpool,
            tc.tile_pool(name="gbuf", bufs=6) as gpool,
            tc.tile_pool(name="ohbuf", bufs=6) as opool,
            tc.tile_pool(name="agg", bufs=1) as apool,
            tc.tile_pool(name="obuf", bufs=4) as obpool,
            tc.tile_pool(name="psum", bufs=6, space="PSUM") as ppool,
            tc.tile_pool(name="psum2", bufs=2, space="PSUM") as ppool2,
        ):
            # ---- idx loads: tiny chunk-0 piece first, then the rest ----
            idx_a0_sb = cpool.tile([128, P0 * 8], I16, tag="idxa0")
            nc.sync.dma_start(out=idx_a0_sb[:], in_=idx_d[:, 0:P0 * 8])
            idx_a_sb = cpool.tile([128, (TA - P0) * 8], I16, tag="idxa")
            nc.sync.dma_start(out=idx_a_sb[:], in_=idx_d[:, P0 * 8:TA * 8])
            idx_b_sb = cpool.tile([128, (TTOT - TA) * 8], I16, tag="idxb")
            nc.sync.dma_start(out=idx_b_sb[:], in_=idx_d[:, TA * 8:TTOT * 8])

            w_sb = cpool.tile([D, D], F32, tag="w")
            nc.sync.dma_start(out=w_sb[:], in_=w_d[:, :])
            brep_sb = cpool.tile([128, D], F32, tag="brep")
            nc.sync.dma_start(out=brep_sb[:], in_=brep_d[:, :])
            ndst_sb = cpool.tile([128, OCH], F32, tag="ndst")
            nc.sync.dma_start(out=ndst_sb[:], in_=ndst_d[:, :])

            aggT = apool.tile([128, NBLK * DST_BLK], F32, tag="aggT")
            nc.vector.memset(aggT[:], 0.0)

            h0 = xh_d[0:HALF, :]
            h1 = xh_d[HALF:RN, :]

            def out_stage(blkid):
                ps2 = ppool2.tile([128, D], F32, tag="ps2")
                nc.tensor.matmul(
                    ps2[:],
                    lhsT=aggT[:, blkid * DST_BLK:(blkid + 1) * DST_BLK],
                    rhs=w_sb[:],
                    start=True,
                    stop=True,
                )
                ob = obpool.tile([128, D], F32, tag="ob")
                if use_bias:
                    nc.vector.tensor_scalar(
                        ob[:], ps2[:], ndst_sb[:, blkid:blkid + 1], None,
                        mybir.AluOpType.mult,
                    )
                    nc.vector.tensor_add(ob[:], ob[:], brep_sb[:])
                    nc.vector.tensor_scalar_max(ob[:], ob[:], 0.0)
                else:
                    nc.scalar.activation(ob[:], ps2[:],
                                         mybir.ActivationFunctionType.Relu,
                                         scale=ndst_sb[:, blkid:blkid + 1])
                nc.sync.dma_start(out=y_d[:, blkid, :], in_=ob[:])

            qn = 0
            for pi, (base, npass, h_ap) in enumerate((
                (0, TA, h0),
                (TA, TTOT - TA, h1),
            )):
                # pass B tapers into small tail pieces on dedicated rings so
                # the final data lands quickly after the last descriptor batch
                sizes = []
                rem = npass
                tail = []
                if pi == 0 and rem >= P0:
                    sizes.append(P0)       # small first call -> fast start
                    rem -= P0
                if pi == 1:
                    while len(tail) < 6 and rem > 8:
                        tail.append(8)
                        rem -= 8
                while rem > 0:
                    s = min(GCH, rem)
                    sizes.append(s)
                    rem -= s
                sizes.extend(reversed(tail))
                psum = None
                t0 = 0
                for ci, nt in enumerate(sizes):
                    nidx = nt * 128
                    a0 = base + t0
                    r0 = t0
                    t0 += nt
                    if pi == 0 and r0 < P0:
                        idx_sb, roff = idx_a0_sb, 0
                    elif pi == 0:
                        idx_sb, roff = idx_a_sb, P0
                    else:
                        idx_sb, roff = idx_b_sb, 0
                    g = gpool.tile([128, GCH, D], BF16, tag="g")
                    if pi == 1 and nt <= 8:
                        qsel = 2 + (qn % 2)       # tail on rings 2/3
                    else:
                        if pi == 1 and ci >= len(sizes) - 6 - 3:
                            qsel = qn % 2          # last bulk on rings 0/1
                        else:
                            qsel = qn % NQ
                    nc.gpsimd.dma_gather(
                        g[:, :nt, :],
                        h_ap,
                        idx_sb[:, (r0 - roff) * 8:(r0 - roff) * 8 + nidx // 16],
                        num_idxs=nidx,
                        num_idxs_reg=nidx,
                        elem_size=D,
                        single_packet=False,
                        queue_num=qsel,
                    )
                    qn += 1
                    oh = opool.tile([128, GCH, DST_BLK], FP8, tag="oh")
                    nc.sync.dma_start(out=oh[:, :nt, :],
                                      in_=oh_d[:, a0:a0 + nt, :])
                    for tl in range(nt):
                        k, T, blkid, halfid = tmeta[a0 + tl]
                        if k == 0:
                            psum = ppool.tile([128, DST_BLK], F32, tag="ps")
                        nc.tensor.matmul(
                            psum[:],
                            lhsT=g[:, tl, :],
                            rhs=oh[:, tl, :],
                            start=(k == 0),
                            stop=(k == T - 1),
                        )
                        if k == T - 1:
                            sl = aggT[:, blkid * DST_BLK:(blkid + 1) * DST_BLK]
                            nc.vector.tensor_add(sl, sl, psum[:])
                            if halfid == 1:
                                out_stage(blkid)

            # blocks with no half-1 tiles never got an out_stage above
            for blkid in range(NBLK):
                if tiles_per_grp[NBLK + blkid] == 0:
                    out_stage(blkid)

    nc.compile()
    return nc


def kernel(x, edge_index, W, b):
    global LAST_RESULTS
    x = np.asarray(x, dtype=np.float32)
    W = np.asarray(W, dtype=np.float32)
    b = np.asarray(b, dtype=np.float32)

    in_maps, tiles_per_grp, TTOT, TA, use_bias = _prep_inputs(
        x, edge_index, W, b)
    nc = _build_program(tiles_per_grp, TTOT, TA, use_bias)

    kwargs = {}
    if TRACE:
        kwargs["trace"] = True
    res = run_bass_kernel_spmd(nc, in_maps, list(range(NCORES)), **kwargs)
    LAST_RESULTS = res

    out = np.empty((N, D), dtype=np.float32)
    for c in range(NCORES):
        yc = np.asarray(res.results[c]["y"])          # [128, OCH, 128]
        rows = yc.transpose(1, 0, 2).reshape(OCH * 128, D)
        out[c * NPC:(c + 1) * NPC] = rows[:NPC]
    return out


# revision 18
# speedup vs baseline: 1.0197x; 1.0197x over previous
"""GCN block (DGL GraphConv norm='both' + ReLU) on 8 TRN2 NeuronCores.

Strategy (SPMD, one program for all cores; per-core data via inputs):
  - Nodes/edges sharded by destination: core c owns dst rows [c*6250, (c+1)*6250).
  - Host folds the source norm into the feature table: xh = x * rsqrt(deg_out)
    cast to bf16 (one 256B row per node). Each core gathers xh[src] for its
    edges straight from its HBM copy with SWDGE dma_gather — no on-device
    table build; gathers start immediately. SWDGE descriptor generation on
    the Q7 (~2.3ns/row marginal, serial) is the kernel's critical path, so
    everything else is kept off it and overlapped under it.
  - Segment-sum by dst via TensorE: edges grouped by (128-wide dst block,
    src half) with a VARIABLE number of 128-edge tiles per group (max over
    cores, so the single SPMD program fits every core's data with ~6% padding
    instead of the ~32% a global max would cost). Per tile one matmul
    (lhsT = gathered rows [128e x 128f] bf16, rhs = is_equal one-hot
    [128e x 128d] bf16) accumulates the block's aggT[f, d] in PSUM; flushes
    add into a memset aggT so empty groups need no special casing.
  - Output stage is pipelined into pass B: as soon as block b's second-half
    flush lands, aggT_b x W (f32 matmul), then ReLU with the rsqrt(deg_in)
    row scale fused on the SCALAR engine (exact: the per-dst scale commutes
    through the feature matmul), DMA out. Nonzero bias falls back to vector.
  - Pass B tapers into small tail gathers on dedicated rings so the final
    rows land right after the last descriptor batch.

dma_gather indices are int16, so the table is addressed in two halves at row
32768; pass A covers half-0 edges, pass B half-1.
"""

import sys

if "/opt/trn_rl_repo" not in sys.path:
    sys.path.insert(0, "/opt/trn_rl_repo")

import numpy as np
import ml_dtypes

import concourse.bacc as bacc
import concourse.mybir as mybir
from concourse.bass_utils import run_bass_kernel_spmd
from concourse.tile import TileContext

N = 50000          # nodes
D = 128            # feature dim
NCORES = 8
NPC = N // NCORES  # 6250 dst nodes per core
RN = 50048         # padded table rows (multiple of 128)
HALF = 32768       # int16 index limit; table addressed [0, HALF) / [HALF, RN)

DST_BLK = 128                     # dst nodes per PSUM block
NBLK = (NPC + DST_BLK - 1) // DST_BLK   # 49
OCH = NBLK                        # output chunks of 128 dst rows

GCH = 32                          # gather chunk: tiles per dma_gather call
NQ = 4                            # SWDGE queues used round-robin

F32 = mybir.dt.float32
BF16 = mybir.dt.bfloat16
FP8 = mybir.dt.float8e4
I16 = mybir.dt.int16

TRACE = False            # set by test harness for profiling
LAST_RESULTS = None      # BassKernelResults of the last run


def _gather_idx_layout(vals):
    """[S] int16 -> [128, S//16] in dma_gather layout (16-wrap, 8x replicated)."""
    base = vals.reshape(-1, 16).T          # [16, S/16]
    return np.ascontiguousarray(np.tile(base, (8, 1)))


def _prep_inputs(x, edge_index, W, b):
    src = np.asarray(edge_index[0], dtype=np.int64)
    dst = np.asarray(edge_index[1], dtype=np.int64)

    deg_out = np.bincount(src, minlength=N).astype(np.float32)
    deg_in = np.bincount(dst, minlength=N).astype(np.float32)
    nsrc = 1.0 / np.sqrt(np.maximum(deg_out, 1.0))
    ndst = 1.0 / np.sqrt(np.maximum(deg_in, 1.0))

    core = dst // NPC
    half = (src >= HALF).astype(np.int64)
    dstl = dst - core * NPC                # local dst id
    blk = dstl // DST_BLK                  # 0..NBLK-1

    # group = (half, blk); edges with the same (core, grp, src) share one
    # gathered row (multi-hot one-hot column), trimming ~2% of the rows
    grp = half * NBLK + blk                # 0..2*NBLK-1
    key = core * (2 * NBLK) + grp
    pairkey = key * N + src
    order = np.argsort(pairkey, kind="stable")
    pk_s = pairkey[order]
    first = np.empty(len(src), dtype=bool)
    first[0] = True
    first[1:] = pk_s[1:] != pk_s[:-1]
    uidx = np.cumsum(first) - 1            # edge (sorted) -> unique-slot index
    ukey = key[order][first]               # per unique slot
    usrc = src[order][first]
    uhalf = half[order][first]
    ucore = core[order][first]
    ugrp = grp[order][first]

    ucounts = np.bincount(ukey, minlength=NCORES * 2 * NBLK).reshape(
        NCORES, 2 * NBLK)
    tiles_per_grp = -(-ucounts.max(axis=0) // 128)      # [2*NBLK] int
    grp_tile_start = np.zeros(2 * NBLK + 1, dtype=np.int64)
    np.cumsum(tiles_per_grp, out=grp_tile_start[1:])
    TTOT = int(grp_tile_start[-1])
    TA = int(tiles_per_grp[:NBLK].sum())   # tiles in pass A (half 0)

    # slot of each unique (core, grp, src): per (core, grp) running rank
    gstart = np.zeros(NCORES * 2 * NBLK + 1, dtype=np.int64)
    np.cumsum(ucounts.reshape(-1), out=gstart[1:])
    urank = np.arange(len(usrc), dtype=np.int64) - gstart[ukey]
    uslot = ucore * (TTOT * 128) + grp_tile_start[ugrp] * 128 + urank

    idx_all = np.zeros(NCORES * TTOT * 128, dtype=np.int16)   # pad: row 0
    idx_all[uslot] = (usrc - uhalf * HALF).astype(np.int16)

    # host multi-hot (small integer counts, exact in fp8):
    # oh[slot, dloc] += 1 per edge
    oh_f32 = np.zeros((NCORES * TTOT * 128, DST_BLK), dtype=np.float32)
    np.add.at(oh_f32, (uslot[uidx], (dstl - blk * DST_BLK)[order]), 1.0)
    oh_all = oh_f32.astype(ml_dtypes.float8_e4m3)
    del oh_f32

    # replicated tensors
    xh = np.zeros((RN, D), dtype=ml_dtypes.bfloat16)
    xh[:N] = (np.asarray(x, dtype=np.float32) * nsrc[:, None]).astype(
        ml_dtypes.bfloat16)

    W_dev = np.ascontiguousarray(np.asarray(W, dtype=np.float32))
    use_bias = bool(np.any(np.asarray(b, dtype=np.float32) != 0.0))
    brep = np.ascontiguousarray(
        np.tile(np.asarray(b, dtype=np.float32)[None, :], (128, 1)))

    in_maps = []
    idx3 = idx_all.reshape(NCORES, TTOT * 128)
    oh4 = oh_all.reshape(NCORES, TTOT, 128, DST_BLK)
    for c in range(NCORES):
        nd = np.ones(OCH * 128, dtype=np.float32)
        nd[:NPC] = ndst[c * NPC:(c + 1) * NPC]
        ndst_dev = np.ascontiguousarray(nd.reshape(OCH, 128).T)
        in_maps.append({
            "xh": xh,
            "w": W_dev,
            "brep": brep,
            "ndst": ndst_dev,
            "idx": _gather_idx_layout(idx3[c]),
            # oh tile t lane e col d -> oh_dev[e, t, d]
            "oh": np.ascontiguousarray(oh4[c].transpose(1, 0, 2)),
        })
    return in_maps, [int(t) for t in tiles_per_grp], TTOT, TA, use_bias


def _build_program(tiles_per_grp, TTOT, TA, use_bias):
    nc = bacc.Bacc("TRN2", target_bir_lowering=False, debug=False,
                   num_devices=NCORES, num_swdge_queues=NQ)

    xh_d = nc.dram_tensor("xh", [RN, D], BF16, kind="ExternalInput")
    w_d = nc.dram_tensor("w", [D, D], F32, kind="ExternalInput")
    brep_d = nc.dram_tensor("brep", [128, D], F32, kind="ExternalInput")
    oh_d = nc.dram_tensor("oh", [128, TTOT, DST_BLK], FP8,
                          kind="ExternalInput")
    ndst_d = nc.dram_tensor("ndst", [128, OCH], F32, kind="ExternalInput")
    idx_d = nc.dram_tensor("idx", [128, TTOT * 8], I16, kind="ExternalInput")
    y_d = nc.dram_tensor("y", [128, OCH, D], F32, kind="ExternalOutput")

    # per-tile metadata: (k within group, group size, blk, half)
    tmeta = []
    for g in range(2 * NBLK):
        T = tiles_per_grp[g]
        for k in range(T):
            tmeta.append((k, T, g % NBLK, g // NBLK))
    assert len(tmeta) == TTOT

    P0 = min(8, TA)    # first idx piece/call: small, so gathers start fast

    with TileContext(nc) as tc:
        with (
            tc.tile_pool(name="const", bufs=1) as cpool,
            tc.tile_pool(name="gbuf", bufs=6) as gpool,
            tc.tile_pool(name="ohbuf", bufs=6) as opool,
            tc.tile_pool(name="agg", bufs=1) as apool,
            tc.tile_pool(name="obuf", bufs=4) as obpool,
            tc.tile_pool(name="psum", bufs=6, space="PSUM") as ppool,
            tc.tile_pool(name="psum2", bufs=2, space="PSUM") as ppool2,
        ):
            # ---- idx loads: tiny chunk-0 piece first, then the rest ----
            idx_a0_sb = cpool.tile([128, P0 * 8], I16, tag="idxa0")
            nc.sync.dma_start(out=idx_a0_sb[:], in_=idx_d[:, 0:P0 * 8])
            idx_a_sb = cpool.tile([128, (TA - P0) * 8], I16, tag="idxa")
            nc.sync.dma_start(out=idx_a_sb[:], in_=idx_d[:, P0 * 8:TA * 8])
            idx_b_sb = cpool.tile([128, (TTOT - TA) * 8], I16, tag="idxb")
            nc.sync.dma_start(out=idx_b_sb[:], in_=idx_d[:, TA * 8:TTOT * 8])

            w_sb = cpool.tile([D, D], F32, tag="w")
            nc.sync.dma_start(out=w_sb[:], in_=w_d[:, :])
            brep_sb = cpool.tile([128, D], F32, tag="brep")
            nc.sync.dma_start(out=brep_sb[:], in_=brep_d[:, :])
            ndst_sb = cpool.tile([128, OCH], F32, tag="ndst")
            nc.sync.dma_start(out=ndst_sb[:], in_=ndst_d[:, :])

            aggT = apool.tile([128, NBLK * DST_BLK], F32, tag="aggT")
            nc.vector.memset(aggT[:], 0.0)

            h0 = xh_d[0:HALF, :]
            h1 = xh_d[HALF:RN, :]

            def out_stage(blkid):
                ps2 = ppool2.tile([128, D], F32, tag="ps2")
                nc.tensor.matmul(
                    ps2[:],
                    lhsT=aggT[:, blkid * DST_BLK:(blkid + 1) * DST_BLK],
                    rhs=w_sb[:],
                    start=True,
                    stop=True,
                )
                ob = obpool.tile([128, D], F32, tag="ob")
                if use_bias:
                    nc.vector.tensor_scalar(
                        ob[:], ps2[:], ndst_sb[:, blkid:blkid + 1], None,
                        mybir.AluOpType.mult,
                    )
                    nc.vector.tensor_add(ob[:], ob[:], brep_sb[:])
                    nc.vector.tensor_scalar_max(ob[:], ob[:], 0.0)
                else:
                    nc.scalar.activation(ob[:], ps2[:],
                                         mybir.ActivationFunctionType.Relu,
                                         scale=ndst_sb[:, blkid:blkid + 1])
                nc.sync.dma_start(out=y_d[:, blkid, :], in_=ob[:])

            qn = 0
            for pi, (base, npass, h_ap) in enumerate((
                (0, TA, h0),
                (TA, TTOT - TA, h1),
            )):
                # pass B tapers into small tail pieces on dedicated rings so
                # the final data lands quickly after the last descriptor batch
                sizes = []
                rem = npass
                tail = []
                if pi == 0 and rem >= P0:
                    sizes.append(P0)       # small first call -> fast start
                    rem -= P0
                if pi == 1:
                    while len(tail) < 6 and rem > 8:
                        tail.append(8)
                        rem -= 8
                while rem > 0:
                    s = min(GCH, rem)
                    sizes.append(s)
                    rem -= s
                sizes.extend(reversed(tail))
                psum = None
                t0 = 0
                for ci, nt in enumerate(sizes):
                    nidx = nt * 128
                    a0 = base + t0
                    r0 = t0
                    t0 += nt
                    if pi == 0 and r0 < P0:
                        idx_sb, roff = idx_a0_sb, 0
                    elif pi == 0:
                        idx_sb, roff = idx_a_sb, P0
                    else:
                        idx_sb, roff = idx_b_sb, 0
                    g = gpool.tile([128, GCH, D], BF16, tag="g")
                    if pi == 1 and nt <= 8:
                        qsel = 2 + (qn % 2)       # tail on rings 2/3
                    else:
                        if pi == 1 and ci >= len(sizes) - 6 - 3:
                            qsel = qn % 2          # last bulk on rings 0/1
                        else:
                            qsel = qn % NQ
                    nc.gpsimd.dma_gather(
                        g[:, :nt, :],
                        h_ap,
                        idx_sb[:, (r0 - roff) * 8:(r0 - roff) * 8 + nidx // 16],
                        num_idxs=nidx,
                        num_idxs_reg=nidx,
                        elem_size=D,
                        single_packet=False,
                        queue_num=qsel,
                    )
                    qn += 1
                    oh = opool.tile([128, GCH, DST_BLK], FP8, tag="oh")
                    nc.sync.dma_start(out=oh[:, :nt, :],
                                      in_=oh_d[:, a0:a0 + nt, :])
                    for tl in range(nt):
                        k, T, blkid, halfid = tmeta[a0 + tl]
                        if k == 0:
                            psum = ppool.tile([128, DST_BLK], F32, tag="ps")
                        nc.tensor.matmul(
                            psum[:],
                            lhsT=g[:, tl, :],
                            rhs=oh[:, tl, :],
                            start=(k == 0),
                            stop=(k == T - 1),
                        )
                        if k == T - 1:
                            sl = aggT[:, blkid * DST_BLK:(blkid + 1) * DST_BLK]
                            nc.vector.tensor_add(sl, sl, psum[:])
                            if halfid == 1:
                                out_stage(blkid)

            # blocks with no half-1 tiles never got an out_stage above
            for blkid in range(NBLK):
                if tiles_per_grp[NBLK + blkid] == 0:
                    out_stage(blkid)

    nc.compile()
    return nc


def kernel(x, edge_index, W, b):
    global LAST_RESULTS
    x = np.asarray(x, dtype=np.float32)
    W = np.asarray(W, dtype=np.float32)
    b = np.asarray(b, dtype=np.float32)

    in_maps, tiles_per_grp, TTOT, TA, use_bias = _prep_inputs(
        x, edge_index, W, b)
    nc = _build_program(tiles_per_grp, TTOT, TA, use_bias)

    kwargs = {}
    if TRACE:
        kwargs["trace"] = True
    res = run_bass_kernel_spmd(nc, in_maps, list(range(NCORES)), **kwargs)
    LAST_RESULTS = res

    out = np.empty((N, D), dtype=np.float32)
    for c in range(NCORES):
        yc = np.asarray(res.results[c]["y"])          # [128, OCH, 128]
        rows = yc.transpose(1, 0, 2).reshape(OCH * 128, D)
        out[c * NPC:(c + 1) * NPC] = rows[:NPC]
    return out


# revision 19
# speedup vs baseline: 1.0932x; 1.0720x over previous
"""GCN block (DGL GraphConv norm='both' + ReLU) on 8 TRN2 NeuronCores.

Strategy (SPMD, one program for all cores; per-core data via inputs):
  - Nodes/edges sharded by destination: core c owns dst rows [c*6250, (c+1)*6250).
  - Host folds the source norm into the feature table: xh = x * rsqrt(deg_out)
    cast to bf16 (one 256B row per node). Each core gathers xh[src] for its
    edges straight from its HBM copy with SWDGE dma_gather — no on-device
    table build; gathers start immediately. SWDGE descriptor generation on
    the Q7 (~2.3ns/row marginal, serial) is the kernel's critical path, so
    everything else is kept off it and overlapped under it.
  - Segment-sum by dst via TensorE: edges grouped by (128-wide dst block,
    src half), deduped by (core, group, src) into multi-hot slots, with a
    VARIABLE number of 128-slot tiles per group (max over cores, so the
    single SPMD program fits every core's data with ~6% padding instead of
    the ~32% a global max would cost). Per tile one matmul (lhsT = gathered
    rows [128e x 128f] bf16, rhs = HOST-BUILT multi-hot [128e x 128d] fp8,
    exact small-integer counts) accumulates the block's aggT[f, d] in PSUM;
    flushes add into a memset aggT so empty groups need no special casing.
    The multi-hot streams in by DMA (13.6MB/core) — keeping the vector
    engine nearly idle measurably speeds up the Q7 descriptor loop.
  - Output stage is pipelined into pass B: as soon as block b's second-half
    flush lands, aggT_b x W (f32 matmul), then ReLU with the rsqrt(deg_in)
    row scale fused on the SCALAR engine (exact: the per-dst scale commutes
    through the feature matmul), DMA out. Nonzero bias falls back to vector.
  - Pass B tapers into small tail gathers on dedicated rings so the final
    rows land right after the last descriptor batch.

dma_gather indices are int16, so the table is addressed in two halves at row
32768; pass A covers half-0 edges, pass B half-1.
"""

import sys

if "/opt/trn_rl_repo" not in sys.path:
    sys.path.insert(0, "/opt/trn_rl_repo")

import numpy as np
import ml_dtypes

import concourse.bacc as bacc
import concourse.mybir as mybir
from concourse.bass_utils import run_bass_kernel_spmd
from concourse.tile import TileContext

N = 50000          # nodes
D = 128            # feature dim
NCORES = 8
NPC = N // NCORES  # 6250 dst nodes per core
RN = 50048         # padded table rows (multiple of 128)
HALF = 32768       # int16 index limit; table addressed [0, HALF) / [HALF, RN)

DST_BLK = 128                     # dst nodes per PSUM block
NBLK = (NPC + DST_BLK - 1) // DST_BLK   # 49
OCH = NBLK                        # output chunks of 128 dst rows

GCH = 32                          # gather chunk: tiles per dma_gather call
NQ = 4                            # SWDGE queues used round-robin

F32 = mybir.dt.float32
BF16 = mybir.dt.bfloat16
FP8 = mybir.dt.float8e4
I16 = mybir.dt.int16

TRACE = False            # set by test harness for profiling
LAST_RESULTS = None      # BassKernelResults of the last run


def _gather_idx_layout(vals):
    """[S] int16 -> [128, S//16] in dma_gather layout (16-wrap, 8x replicated)."""
    base = vals.reshape(-1, 16).T          # [16, S/16]
    return np.ascontiguousarray(np.tile(base, (8, 1)))


def _prep_inputs(x, edge_index, W, b):
    src = np.asarray(edge_index[0], dtype=np.int64)
    dst = np.asarray(edge_index[1], dtype=np.int64)

    deg_out = np.bincount(src, minlength=N).astype(np.float32)
    deg_in = np.bincount(dst, minlength=N).astype(np.float32)
    nsrc = 1.0 / np.sqrt(np.maximum(deg_out, 1.0))
    ndst = 1.0 / np.sqrt(np.maximum(deg_in, 1.0))

    core = dst // NPC
    half = (src >= HALF).astype(np.int64)
    dstl = dst - core * NPC                # local dst id
    blk = dstl // DST_BLK                  # 0..NBLK-1

    # group = (half, blk); edges with the same (core, grp, src) share one
    # gathered row (multi-hot one-hot column), trimming ~2% of the rows
    grp = half * NBLK + blk                # 0..2*NBLK-1
    key = core * (2 * NBLK) + grp
    pairkey = key * N + src
    order = np.argsort(pairkey, kind="stable")
    pk_s = pairkey[order]
    first = np.empty(len(src), dtype=bool)
    first[0] = True
    first[1:] = pk_s[1:] != pk_s[:-1]
    uidx = np.cumsum(first) - 1            # edge (sorted) -> unique-slot index
    ukey = key[order][first]               # per unique slot
    usrc = src[order][first]
    uhalf = half[order][first]
    ucore = core[order][first]
    ugrp = grp[order][first]

    ucounts = np.bincount(ukey, minlength=NCORES * 2 * NBLK).reshape(
        NCORES, 2 * NBLK)
    tiles_per_grp = -(-ucounts.max(axis=0) // 128)      # [2*NBLK] int
    grp_tile_start = np.zeros(2 * NBLK + 1, dtype=np.int64)
    np.cumsum(tiles_per_grp, out=grp_tile_start[1:])
    TTOT = int(grp_tile_start[-1])
    TA = int(tiles_per_grp[:NBLK].sum())   # tiles in pass A (half 0)

    # slot of each unique (core, grp, src): per (core, grp) running rank
    gstart = np.zeros(NCORES * 2 * NBLK + 1, dtype=np.int64)
    np.cumsum(ucounts.reshape(-1), out=gstart[1:])
    urank = np.arange(len(usrc), dtype=np.int64) - gstart[ukey]
    uslot = ucore * (TTOT * 128) + grp_tile_start[ugrp] * 128 + urank

    idx_all = np.zeros(NCORES * TTOT * 128, dtype=np.int16)   # pad: row 0
    idx_all[uslot] = (usrc - uhalf * HALF).astype(np.int16)

    # host multi-hot (small integer counts, exact in fp8):
    # oh[slot, dloc] += 1 per edge
    oh_f32 = np.zeros((NCORES * TTOT * 128, DST_BLK), dtype=np.float32)
    np.add.at(oh_f32, (uslot[uidx], (dstl - blk * DST_BLK)[order]), 1.0)
    oh_all = oh_f32.astype(ml_dtypes.float8_e4m3)
    del oh_f32

    # replicated tensors
    xh = np.zeros((RN, D), dtype=ml_dtypes.bfloat16)
    xh[:N] = (np.asarray(x, dtype=np.float32) * nsrc[:, None]).astype(
        ml_dtypes.bfloat16)

    W_dev = np.ascontiguousarray(np.asarray(W, dtype=np.float32))
    use_bias = bool(np.any(np.asarray(b, dtype=np.float32) != 0.0))
    brep = np.ascontiguousarray(
        np.tile(np.asarray(b, dtype=np.float32)[None, :], (128, 1)))

    in_maps = []
    idx3 = idx_all.reshape(NCORES, TTOT * 128)
    oh4 = oh_all.reshape(NCORES, TTOT, 128, DST_BLK)
    for c in range(NCORES):
        nd = np.ones(OCH * 128, dtype=np.float32)
        nd[:NPC] = ndst[c * NPC:(c + 1) * NPC]
        ndst_dev = np.ascontiguousarray(nd.reshape(OCH, 128).T)
        in_maps.append({
            "xh": xh,
            "w": W_dev,
            "brep": brep,
            "ndst": ndst_dev,
            "idx": _gather_idx_layout(idx3[c]),
            # oh tile t lane e col d -> oh_dev[e, t, d]
            "oh": np.ascontiguousarray(oh4[c].transpose(1, 0, 2)),
        })
    return in_maps, [int(t) for t in tiles_per_grp], TTOT, TA, use_bias


def _build_program(tiles_per_grp, TTOT, TA, use_bias):
    nc = bacc.Bacc("TRN2", target_bir_lowering=False, debug=False,
                   num_devices=NCORES, num_swdge_queues=NQ)

    xh_d = nc.dram_tensor("xh", [RN, D], BF16, kind="ExternalInput")
    w_d = nc.dram_tensor("w", [D, D], F32, kind="ExternalInput")
    brep_d = nc.dram_tensor("brep", [128, D], F32, kind="ExternalInput")
    oh_d = nc.dram_tensor("oh", [128, TTOT, DST_BLK], FP8,
                          kind="ExternalInput")
    ndst_d = nc.dram_tensor("ndst", [128, OCH], F32, kind="ExternalInput")
    idx_d = nc.dram_tensor("idx", [128, TTOT * 8], I16, kind="ExternalInput")
    y_d = nc.dram_tensor("y", [128, OCH, D], F32, kind="ExternalOutput")

    # per-tile metadata: (k within group, group size, blk, half)
    tmeta = []
    for g in range(2 * NBLK):
        T = tiles_per_grp[g]
        for k in range(T):
            tmeta.append((k, T, g % NBLK, g // NBLK))
    assert len(tmeta) == TTOT

    P0 = min(8, TA)    # first idx piece/call: small, so gathers start fast

    with TileContext(nc) as tc:
        with (
            tc.tile_pool(name="const", bufs=1) as cpool,
            tc.tile_pool(name="gbuf", bufs=6) as gpool,
            tc.tile_pool(name="ohbuf", bufs=6) as opool,
            tc.tile_pool(name="agg", bufs=1) as apool,
            tc.tile_pool(name="obuf", bufs=4) as obpool,
            tc.tile_pool(name="psum", bufs=6, space="PSUM") as ppool,
            tc.tile_pool(name="psum2", bufs=2, space="PSUM") as ppool2,
        ):
            # ---- idx loads: tiny chunk-0 piece first, then the rest ----
            idx_a0_sb = cpool.tile([128, P0 * 8], I16, tag="idxa0")
            nc.sync.dma_start(out=idx_a0_sb[:], in_=idx_d[:, 0:P0 * 8])
            idx_a_sb = cpool.tile([128, (TA - P0) * 8], I16, tag="idxa")
            nc.sync.dma_start(out=idx_a_sb[:], in_=idx_d[:, P0 * 8:TA * 8])
            idx_b_sb = cpool.tile([128, (TTOT - TA) * 8], I16, tag="idxb")
            nc.sync.dma_start(out=idx_b_sb[:], in_=idx_d[:, TA * 8:TTOT * 8])

            w_sb = cpool.tile([D, D], F32, tag="w")
            nc.sync.dma_start(out=w_sb[:], in_=w_d[:, :])
            brep_sb = cpool.tile([128, D], F32, tag="brep")
            nc.sync.dma_start(out=brep_sb[:], in_=brep_d[:, :])
            ndst_sb = cpool.tile([128, OCH], F32, tag="ndst")
            nc.sync.dma_start(out=ndst_sb[:], in_=ndst_d[:, :])

            aggT = apool.tile([128, NBLK * DST_BLK], F32, tag="aggT")
            nc.vector.memset(aggT[:], 0.0)

            h0 = xh_d[0:HALF, :]
            h1 = xh_d[HALF:RN, :]

            def out_stage(blkid):
                ps2 = ppool2.tile([128, D], F32, tag="ps2")
                nc.tensor.matmul(
                    ps2[:],
                    lhsT=aggT[:, blkid * DST_BLK:(blkid + 1) * DST_BLK],
                    rhs=w_sb[:],
                    start=True,
                    stop=True,
                )
                ob = obpool.tile([128, D], F32, tag="ob")
                if use_bias:
                    nc.vector.tensor_scalar(
                        ob[:], ps2[:], ndst_sb[:, blkid:blkid + 1], None,
                        mybir.AluOpType.mult,
                    )
                    nc.vector.tensor_add(ob[:], ob[:], brep_sb[:])
                    nc.vector.tensor_scalar_max(ob[:], ob[:], 0.0)
                else:
                    nc.scalar.activation(ob[:], ps2[:],
                                         mybir.ActivationFunctionType.Relu,
                                         scale=ndst_sb[:, blkid:blkid + 1])
                nc.sync.dma_start(out=y_d[:, blkid, :], in_=ob[:])

            qn = 0
            for pi, (base, npass, h_ap) in enumerate((
                (0, TA, h0),
                (TA, TTOT - TA, h1),
            )):
                # pass B tapers into small tail pieces on dedicated rings so
                # the final data lands quickly after the last descriptor batch
                sizes = []
                rem = npass
                tail = []
                if pi == 0 and rem >= P0:
                    sizes.append(P0)       # small first call -> fast start
                    rem -= P0
                if pi == 1:
                    while len(tail) < 6 and rem > 8:
                        tail.append(8)
                        rem -= 8
                while rem > 0:
                    s = min(GCH, rem)
                    sizes.append(s)
                    rem -= s
                sizes.extend(reversed(tail))
                psum = None
                t0 = 0
                for ci, nt in enumerate(sizes):
                    nidx = nt * 128
                    a0 = base + t0
                    r0 = t0
                    t0 += nt
                    if pi == 0 and r0 < P0:
                        idx_sb, roff = idx_a0_sb, 0
                    elif pi == 0:
                        idx_sb, roff = idx_a_sb, P0
                    else:
                        idx_sb, roff = idx_b_sb, 0
                    g = gpool.tile([128, GCH, D], BF16, tag="g")
                    if pi == 1 and nt <= 8:
                        qsel = 2 + (qn % 2)       # tail on rings 2/3
                    else:
                        if pi == 1 and ci >= len(sizes) - 6 - 3:
                            qsel = qn % 2          # last bulk on rings 0/1
                        else:
                            qsel = qn % NQ
                    nc.gpsimd.dma_gather(
                        g[:, :nt, :],
                        h_ap,
                        idx_sb[:, (r0 - roff) * 8:(r0 - roff) * 8 + nidx // 16],
                        num_idxs=nidx,
                        num_idxs_reg=nidx,
                        elem_size=D,
                        single_packet=False,
                        queue_num=qsel,
                    )
                    qn += 1
                    oh = opool.tile([128, GCH, DST_BLK], FP8, tag="oh")
                    nc.sync.dma_start(out=oh[:, :nt, :],
                                      in_=oh_d[:, a0:a0 + nt, :])
                    for tl in range(nt):
                        k, T, blkid, halfid = tmeta[a0 + tl]
                        if k == 0:
                            psum = ppool.tile([128, DST_BLK], F32, tag="ps")
                        nc.tensor.matmul(
                            psum[:],
                            lhsT=g[:, tl, :],
                            rhs=oh[:, tl, :],
                            start=(k == 0),
                            stop=(k == T - 1),
                        )
                        if k == T - 1:
                            sl = aggT[:, blkid * DST_BLK:(blkid + 1) * DST_BLK]
                            nc.vector.tensor_add(sl, sl, psum[:])
                            if halfid == 1:
                                out_stage(blkid)

            # blocks with no half-1 tiles never got an out_stage above
            for blkid in range(NBLK):
                if tiles_per_grp[NBLK + blkid] == 0:
                    out_stage(blkid)

    nc.compile()
    return nc


def kernel(x, edge_index, W, b):
    global LAST_RESULTS
    x = np.asarray(x, dtype=np.float32)
    W = np.asarray(W, dtype=np.float32)
    b = np.asarray(b, dtype=np.float32)

    in_maps, tiles_per_grp, TTOT, TA, use_bias = _prep_inputs(
        x, edge_index, W, b)
    nc = _build_program(tiles_per_grp, TTOT, TA, use_bias)

    kwargs = {}
    if TRACE:
        kwargs["trace"] = True
    res = run_bass_kernel_spmd(nc, in_maps, list(range(NCORES)), **kwargs)
    LAST_RESULTS = res

    out = np.empty((N, D), dtype=np.float32)
    for c in range(NCORES):
        yc = np.asarray(res.results[c]["y"])          # [128, OCH, 128]
        rows = yc.transpose(1, 0, 2).reshape(OCH * 128, D)
        out[c * NPC:(c + 1) * NPC] = rows[:NPC]
    return out
